# revision 30
# baseline (speedup 1.0000x reference)
"""Distributed self-attention kernel for one TRN2 chip (8 NeuronCores).

Problem: b=2, n=2048, d=1024, 16 heads x 64 dim, fp32 in/out.

Sharding (per the hint: data-parallel on b, tensor-parallel on h):
  core i -> batch b = i//4, head group g = i%4 (heads 4g..4g+3).
  Each core projects Q/K/V for its 4 heads from the full sequence of its
  batch, runs attention, and applies its two contiguous 128-row blocks of Wo
  to produce partial (n, d) outputs; kernel() sums the partials per batch
  while unsharding (tensor-parallel out-projection with the reduction folded
  into the host-side unshard: an on-device AllGather/AllReduce costs 25-45us
  of tail latency, most of it inter-core skew wait at the sync point).

Device layout notes:
  - tokens arrive pre-transposed and pre-tiled [ic, p, dk, 512] so each of
    the 4 quarter-sequence DMAs is 128 contiguous 8KB descriptors (one per
    partition); weights are tiled the same way ([p, dk, c]). All input DMAs
    are issued from the sync sequencer in dependency-priority order (wk and
    the first token quarter first, split in halves) so the first projection
    matmul starts ~11us after launch (7us of that is fixed sequencer
    preamble) instead of ~17us.
  - matmul operands are bf16 (PSUM accumulation is fp32). fp8 was measured:
    the PE streams one moving column per cycle regardless, so fp8 DoubleRow
    only helps when 256 contraction rows can share one instruction - never
    true here (sim contracts 64, AV needs exact bf16 weights).
  - sim is computed transposed (j on partitions, i free) so that softmax'd
    tiles feed the AV matmul with no transpose; softmax denominators come
    from a ones-column appended to V (65th output partition of the AV psum).
  - exp runs on ScalarE over 1024-wide psum tiles (2 banks) to amortize the
    per-instruction PSUM-access overhead (~1.1us per 128x1024 tile; under
    the ~1.5us of PE work per key tile, so ScalarE never paces).
  - Wo partials go straight from PSUM->SBUF->DRAM per (pair, 1024-col seq
    chunk) as soon as both heads of the pair finish that chunk; the two
    pair-partials are separate DRAM tensors summed on the host, which costs
    8MB of extra DMA but removes 64KB/partition of SBUF staging and the
    pair0->pair1 ordering constraint.
  - context_mask is all-ones by construction (spec fill=ones) and is ignored.
"""

import sys

if "/opt/trn_rl_repo" not in sys.path:
    sys.path.append("/opt/trn_rl_repo")

import ml_dtypes
import numpy as np

import concourse.bass as bass
import concourse.tile as tile
from concourse.tile import add_dep_helper
from concourse import bacc, mybir
from concourse.bass_utils import run_bass_kernel_spmd

F32 = mybir.dt.float32
BF16 = mybir.dt.bfloat16
AF = mybir.ActivationFunctionType
NPBF16 = ml_dtypes.bfloat16

P = 128          # SBUF partitions
B = 2            # batch
N = 2048         # sequence length
D = 1024         # model dim
H = 16           # heads
HD = 64          # head dim
NCORES = 8
G = 4            # cores per batch (replica group size)
HPC = H // G     # heads per core = 4
C = HPC * HD     # per-core inner dim slice = 256
IC = 512         # psum free-dim chunk (one bank)
IC2 = 1024       # exp batch chunk (two banks)
NIC = N // IC    # 4
NIC2 = N // IC2  # 2
JT = N // P      # 16 key tiles
DK = D // P      # 8 contraction chunks

_compiled = {}


def _emit(tc):
    nc = tc.nc
    tok_e = nc.dram_tensor("tok", [NIC, P, DK, IC], BF16, kind="ExternalInput")
    wq_e = nc.dram_tensor("wq", [P, DK, C], BF16, kind="ExternalInput")
    wk_e = nc.dram_tensor("wk", [P, DK, C], BF16, kind="ExternalInput")
    wv_e = nc.dram_tensor("wv", [P, DK, C], BF16, kind="ExternalInput")
    wo_e = nc.dram_tensor("wo", [P, 2, D], BF16, kind="ExternalInput")
    # two partial outputs (one per head-pair); host sums them while unsharding
    out_e = nc.dram_tensor("out", [N, D], F32, kind="ExternalOutput")
    out2_e = nc.dram_tensor("out2", [N, D], F32, kind="ExternalOutput")

    from contextlib import ExitStack

    with ExitStack() as ctx:
        ps_mm = ctx.enter_context(tc.tile_pool(name="ps_mm", bufs=2, space="PSUM"))
        ps_sim = ctx.enter_context(tc.tile_pool(name="ps_sim", bufs=2, space="PSUM"))
        ps_av = ctx.enter_context(tc.tile_pool(name="ps_av", bufs=2, space="PSUM"))
        qk_pool = ctx.enter_context(tc.tile_pool(name="qk", bufs=1))
        v_pool = ctx.enter_context(tc.tile_pool(name="v", bufs=1))
        exp_pool = ctx.enter_context(tc.tile_pool(name="exp", bufs=8))
        attn_pool = ctx.enter_context(tc.tile_pool(name="attnT", bufs=4))
        small = ctx.enter_context(tc.tile_pool(name="small", bufs=4))
        out_pool = ctx.enter_context(tc.tile_pool(name="osb", bufs=4))
        tokp = ctx.enter_context(tc.tile_pool(name="tok", bufs=1))
        wp = ctx.enter_context(tc.tile_pool(name="w", bufs=1))

        # ---- input DMA: few fat transfers (one descriptor per partition),
        # all on the sync sequencer so queue order == priority order ----
        tok = [tokp.tile([P, DK, IC], BF16, tag=f"tok{ic}", name=f"tok{ic}")
               for ic in range(NIC)]
        wq_sb = wp.tile([P, DK, C], BF16, tag="wq", name="wqs")
        wk_sb = wp.tile([P, DK, C], BF16, tag="wk", name="wks")
        wv_sb = wp.tile([P, DK, C], BF16, tag="wv", name="wvs")
        wo_sb = wp.tile([P, 2, D], BF16, tag="wo", name="wos")

        # all input DMAs go through the sync sequencer: descriptors enqueue
        # in trigger order, so issuing from one engine in priority order
        # guarantees the critical path (wk + first token quarter) drains the
        # queues first. (Issuing the rest from scalar/gpsimd in parallel was
        # tried and regressed: their descriptors jump ahead of tok0's.)
        nc.sync.dma_start(out=wk_sb[:, 0:4, :], in_=wk_e[:, 0:4, :])
        nc.sync.dma_start(out=tok[0][:, 0:4, :], in_=tok_e[0, :, 0:4, :])
        nc.sync.dma_start(out=wk_sb[:, 4:8, :], in_=wk_e[:, 4:8, :])
        nc.sync.dma_start(out=tok[0][:, 4:8, :], in_=tok_e[0, :, 4:8, :])
        nc.sync.dma_start(out=wq_sb[:], in_=wq_e[:])
        nc.sync.dma_start(out=tok[1][:], in_=tok_e[1])
        nc.sync.dma_start(out=tok[2][:], in_=tok_e[2])
        nc.sync.dma_start(out=tok[3][:], in_=tok_e[3])
        nc.sync.dma_start(out=wv_sb[:], in_=wv_e[:])
        nc.sync.dma_start(out=wo_sb[:], in_=wo_e[:])

        # qT/kT[p]: rows 0-63 head 2p, rows 64-127 head 2p+1
        qT = [qk_pool.tile([P, N], BF16, tag=f"qT{p}", name=f"qT{p}")
              for p in range(2)]
        kT = [qk_pool.tile([P, N], BF16, tag=f"kT{p}", name=f"kT{p}")
              for p in range(2)]
        vtile = v_pool.tile([P, JT, HPC, HD + 1], BF16, tag="v", name="vtile")
        attnT = [[attn_pool.tile([P, IC2], BF16, tag=f"attnT{p}_{c2}",
                                 name=f"attnT{p}_{c2}")
                  for c2 in range(NIC2)] for p in range(2)]

        bg = []  # deferred PE work (one matmul per thunk), drained in attn loops
        last_mm = [None]  # last attention matmul, for tail-wo ordering deps

        def drain_bg(n):
            for _ in range(n):
                if bg:
                    bg.pop(0)()

        def emit_proj(p, ic, w_sb, dst, defer=False):
            state = {}

            def mk(dk, state):
                def thunk():
                    if dk == 0:
                        state["ps"] = ps_mm.tile([P, IC], F32, tag="mm", name="ps")
                    nc.tensor.matmul(
                        state["ps"][:],
                        lhsT=w_sb[:, dk, P * p:P * (p + 1)],
                        rhs=tok[ic][:, dk, :],
                        start=(dk == 0),
                        stop=(dk == DK - 1),
                    )
                    if dk == DK - 1:
                        nc.vector.tensor_copy(
                            dst[:, IC * ic:IC * (ic + 1)], state["ps"][:])
                return thunk

            for dk in range(DK):
                t = mk(dk, state)
                if defer:
                    bg.append(t)
                else:
                    t()

        def emit_v(jt, defer=False):
            state = {}

            def mk_v(dk, state):
                def thunk():
                    if dk == 0:
                        state["ps"] = ps_mm.tile(
                            [P, HPC, HD], F32, tag="mm", name="ps")
                    nc.tensor.matmul(
                        state["ps"][:],
                        lhsT=tok[jt // 4][:, dk, P * (jt % 4):P * (jt % 4 + 1)],
                        rhs=wv_sb[:, dk, :],
                        start=(dk == 0),
                        stop=(dk == DK - 1),
                    )
                    if dk == DK - 1:
                        nc.vector.tensor_copy(
                            vtile[:, jt, :, 0:HD], state["ps"][:])
                return thunk

            for dk in range(DK):
                t = mk_v(dk, state)
                if defer:
                    bg.append(t)
                else:
                    t()

        def emit_attn(p, c2, q, drain=4, pre_norm_cb=None, half_cb=None):
            h = 2 * p + q  # local head index 0..3
            r0 = HD * q    # partition row base inside the pair tiles
            # two av accumulators, one per 512-wide half of this chunk
            avp = [ps_av.tile([HD + 1, IC], F32, tag="av", name="avp")
                   for _ in range(2)]
            ets = [None] * JT

            def av_mm(k, stop):
                for half in range(2):
                    mm = nc.tensor.matmul(
                        avp[half][:],
                        lhsT=vtile[:, k, h, :],
                        rhs=ets[k][:, IC * half:IC * (half + 1)],
                        start=(k == 0),
                        stop=stop,
                    )
                    last_mm[0] = mm
                ets[k] = None

            for jt in range(JT):
                sp = ps_sim.tile([P, IC2], F32, tag="sim", name="sp")
                # two F=512 matmuls: a single F=1024 psum write spanning two
                # banks is rejected by codegen
                for half in range(2):
                    nc.tensor.matmul(
                        sp[:, IC * half:IC * (half + 1)],
                        lhsT=kT[p][r0:r0 + HD, P * jt:P * (jt + 1)],
                        rhs=qT[p][r0:r0 + HD,
                                  IC2 * c2 + IC * half:IC2 * c2 + IC * (half + 1)],
                        start=True,
                        stop=True,
                    )
                et = exp_pool.tile([P, IC2], BF16, tag="exp", name="et")
                nc.scalar.activation(et[:], sp[:], AF.Exp)
                ets[jt] = et
                drain_bg(drain)
                if jt >= 3:
                    av_mm(jt - 3, stop=False)
            av_mm(JT - 3, stop=False)
            av_mm(JT - 2, stop=False)
            av_mm(JT - 1, stop=True)
            if pre_norm_cb is not None:
                pre_norm_cb()
            for half in range(2):
                # reciprocal_approx_fast cannot read PSUM (wrong results were
                # measured) - stage the ones-row sums through SBUF first
                sums = small.tile([1, IC], F32, tag="sums", name="sums")
                nc.vector.tensor_copy(sums[:], avp[half][HD:HD + 1, :])
                rec1 = small.tile([1, IC], F32, tag="rec1", name="rec1")
                nc.vector.reciprocal_approx_fast(out=rec1[:], in_=sums[:])
                rec64 = small.tile([HD, IC], F32, tag="rec64", name="rec64")
                nc.gpsimd.partition_broadcast(rec64[:], rec1[:])
                nc.vector.tensor_mul(
                    attnT[p][c2][r0:r0 + HD, IC * half:IC * (half + 1)],
                    avp[half][0:HD, :],
                    rec64[:],
                )
                if half_cb is not None:
                    half_cb(half)

        def emit_wo(p, c2, i, defer=True, use_alt_psum=False, copy_eng=None):
            # wo partial for pair p, seq tile i (128 rows) of chunk c2,
            # written straight to this pair's dram partial. copy_eng lets
            # tail thunks route their psum->sbuf copies to gpsimd so they
            # never queue ahead of the softmax-normalize chain on DVE.
            dst = out_e if p == 0 else out2_e
            nt = 8 * c2 + i
            if copy_eng == "scalar":
                copier = nc.scalar.copy
            else:
                copier = nc.vector.tensor_copy

            def thunk():
                pss = []
                for do in range(2):
                    if use_alt_psum and do == 1:
                        ps = ps_sim.tile([P, IC], F32, tag="sim", name="ps")
                    else:
                        ps = ps_mm.tile([P, IC], F32, tag="mm", name="ps")
                    mm = nc.tensor.matmul(
                        ps[:],
                        lhsT=attnT[p][c2][:, P * i:P * (i + 1)],
                        rhs=wo_sb[:, p, IC * do:IC * (do + 1)],
                        start=True,
                        stop=True,
                    )
                    if last_mm[0] is not None:
                        add_dep_helper(
                            mm.ins, last_mm[0].ins, sync=False,
                            reason="keep wo behind attention in PE order")
                    pss.append(ps)
                osb = out_pool.tile([P, D], F32, tag="osb", name="osb")
                for do in range(2):
                    copier(osb[:, IC * do:IC * (do + 1)], pss[do][:])
                nc.sync.dma_start(out=dst[P * nt:P * (nt + 1), :], in_=osb[:])

            if defer:
                bg.append(thunk)
            else:
                thunk()

        # ---- emission schedule ----
        # lead-in ordered to match DMA arrival: K ic0 (wk+tok0), Q ic0 (wq),
        # then K/Q as later token quarters land. Attention pair0/c2=0 needs
        # all of K pair0 + Q pair0 chunks 0-1; everything else is deferred
        # into the attention drain loops.
        emit_proj(0, 0, wk_sb, kT[0])
        emit_proj(0, 0, wq_sb, qT[0])
        emit_proj(0, 1, wk_sb, kT[0])
        emit_proj(0, 1, wq_sb, qT[0])
        emit_proj(0, 2, wk_sb, kT[0])
        emit_proj(0, 3, wk_sb, kT[0])
        emit_proj(0, 2, wq_sb, qT[0], defer=True)
        emit_proj(0, 3, wq_sb, qT[0], defer=True)

        nc.vector.memset(vtile[:, :, :, HD:HD + 1], 1.0)
        for jt in range(JT):
            emit_v(jt, defer=(jt >= 3))

        for ic in range(NIC):
            emit_proj(1, ic, wk_sb, kT[1], defer=True)
        for ic in range(NIC):
            emit_proj(1, ic, wq_sb, qT[1], defer=True)

        # drain=10 in the first phase keeps V-tile production ahead of the
        # lag-3 AV consumer.
        emit_attn(0, 0, 0, drain=10)
        emit_attn(0, 0, 1, drain=3)
        for i in range(8):
            emit_wo(0, 0, i)
        emit_attn(0, 1, 0, drain=3)
        emit_attn(0, 1, 1, drain=3)
        for i in range(8):
            emit_wo(0, 1, i)
        emit_attn(1, 0, 0, drain=3)
        emit_attn(1, 0, 1, drain=3)
        for i in range(4):
            emit_wo(1, 0, i)
        emit_attn(1, 1, 0, drain=3)

        # last phase: wo(1,0) tiles 4-7 were held back as ready-to-run PE
        # work to fill the softmax-chain latency after the final AV, and
        # each half's wo(1,1) tiles start as soon as its normalize lands.
        # Tail copies that precede a later DVE normalize go via the scalar
        # engine (idle once the last exp is done) to keep DVE clear.
        def pre_norm():
            drain_bg(len(bg))
            for i in range(4, 8):
                emit_wo(1, 0, i, defer=False, copy_eng="scalar")

        def last_half(half):
            ce = "scalar" if half == 0 else None
            for i in range(4 * half, 4 * half + 4):
                emit_wo(1, 1, i, defer=False, use_alt_psum=True, copy_eng=ce)

        emit_attn(1, 1, 1, drain=2, pre_norm_cb=pre_norm, half_cb=last_half)


def build():
    if "nc" not in _compiled:
        nc = bacc.Bacc("TRN2", target_bir_lowering=False, debug=False,
                       num_devices=NCORES)
        with tile.TileContext(nc) as tc:
            _emit(tc)
        nc.compile()
        _compiled["nc"] = nc
    return _compiled["nc"]


def kernel(tokens, context_mask, Wq, Wkv, Wo, _profile=False):
    tokens = np.asarray(tokens, dtype=np.float32)
    Wq = np.asarray(Wq, dtype=np.float32)
    Wkv = np.asarray(Wkv, dtype=np.float32)
    Wo = np.asarray(Wo, dtype=np.float32)

    nc = build()
    scale = np.float32(HD ** -0.5)

    # tokens[b].T tiled to [ic, p, dk, 512] so each quarter-chunk DMA is one
    # fat contiguous descriptor per partition.
    tokH = []
    for b in range(B):
        t4 = np.ascontiguousarray(tokens[b].T).astype(NPBF16)
        t4 = t4.reshape(DK, P, NIC, IC).transpose(2, 1, 0, 3)
        tokH.append(np.ascontiguousarray(t4))

    def wtile(w):  # [D, C] -> [p, dk, c]
        return np.ascontiguousarray(
            w.astype(NPBF16).reshape(DK, P, C).transpose(1, 0, 2))

    in_maps = []
    for core in range(NCORES):
        b, g = divmod(core, G)
        wo_slice = Wo[C * g:C * (g + 1), :].astype(NPBF16)
        in_maps.append({
            "tok": tokH[b],
            "wq": wtile(Wq[:, C * g:C * (g + 1)] * scale),
            "wk": wtile(Wkv[:, C * g:C * (g + 1)]),
            "wv": wtile(Wkv[:, D + C * g:D + C * (g + 1)]),
            "wo": np.ascontiguousarray(
                wo_slice.reshape(2, P, D).transpose(1, 0, 2)),
        })
    kwargs = {}
    if _profile:
        kwargs = dict(trace=True,
                      tmpdir=_profile if isinstance(_profile, str) else None)
    res = run_bass_kernel_spmd(nc, in_maps, core_ids=list(range(NCORES)), **kwargs)

    out = np.zeros((B, N, D), dtype=np.float32)
    for core in range(NCORES):
        b = core // G
        out[b] += res.results[core]["out"].astype(np.float32)
        out[b] += res.results[core]["out2"].astype(np.float32)
    if _profile:
        return out, res
    return out


# revision 33
# speedup vs baseline: 1.0313x; 1.0313x over previous
"""Distributed self-attention kernel for one TRN2 chip (8 NeuronCores).

Problem: b=2, n=2048, d=1024, 16 heads x 64 dim, fp32 in/out.

Sharding (per the hint: data-parallel on b, tensor-parallel on h):
  core i -> batch b = i//4, head group g = i%4 (heads 4g..4g+3).
  Each core projects Q/K/V for its 4 heads from the full sequence of its
  batch, runs attention, and applies its two contiguous 128-row blocks of Wo
  to produce partial (n, d) outputs; kernel() sums the partials per batch
  while unsharding (tensor-parallel out-projection with the reduction folded
  into the host-side unshard: an on-device AllGather/AllReduce costs 25-45us
  of tail latency, most of it inter-core skew wait at the sync point).

Device layout notes:
  - tokens arrive pre-transposed and pre-tiled [ic, p, dk, 512] so each of
    the 4 quarter-sequence DMAs is 128 contiguous 8KB descriptors (one per
    partition); weights are tiled the same way ([p, dk, c]). All input DMAs
    are issued from the sync sequencer in dependency-priority order (wk and
    the first token quarter first, split in halves) so the first projection
    matmul starts ~11us after launch (7us of that is fixed sequencer
    preamble) instead of ~17us.
  - matmul operands are bf16 (PSUM accumulation is fp32). fp8 was measured:
    the PE streams one moving column per cycle regardless, so fp8 DoubleRow
    only helps when 256 contraction rows can share one instruction - never
    true here (sim contracts 64, AV needs exact bf16 weights).
  - sim is computed transposed (j on partitions, i free) so that softmax'd
    tiles feed the AV matmul with no transpose; softmax denominators come
    from a ones-column appended to V (65th output partition of the AV psum).
  - exp runs on ScalarE over 1024-wide psum tiles (2 banks) to amortize the
    per-instruction PSUM-access overhead (~1.1us per 128x1024 tile; under
    the ~1.5us of PE work per key tile, so ScalarE never paces).
  - Wo partials go straight from PSUM->SBUF->DRAM per (pair, 1024-col seq
    chunk) as soon as both heads of the pair finish that chunk; the two
    pair-partials are separate DRAM tensors summed on the host, which costs
    8MB of extra DMA but removes 64KB/partition of SBUF staging and the
    pair0->pair1 ordering constraint.
  - context_mask is all-ones by construction (spec fill=ones) and is ignored.
"""

import sys

if "/opt/trn_rl_repo" not in sys.path:
    sys.path.append("/opt/trn_rl_repo")

import ml_dtypes
import numpy as np

import concourse.bass as bass
import concourse.tile as tile
from concourse.tile import add_dep_helper
from concourse import bacc, mybir
from concourse.bass_utils import run_bass_kernel_spmd

F32 = mybir.dt.float32
BF16 = mybir.dt.bfloat16
AF = mybir.ActivationFunctionType
NPBF16 = ml_dtypes.bfloat16

P = 128          # SBUF partitions
B = 2            # batch
N = 2048         # sequence length
D = 1024         # model dim
H = 16           # heads
HD = 64          # head dim
NCORES = 8
G = 4            # cores per batch (replica group size)
HPC = H // G     # heads per core = 4
C = HPC * HD     # per-core inner dim slice = 256
IC = 512         # psum free-dim chunk (one bank)
IC2 = 1024       # exp batch chunk (two banks)
NIC = N // IC    # 4
NIC2 = N // IC2  # 2
JT = N // P      # 16 key tiles
DK = D // P      # 8 contraction chunks

_compiled = {}


def _emit(tc):
    nc = tc.nc
    tok_e = nc.dram_tensor("tok", [NIC, P, DK, IC], BF16, kind="ExternalInput")
    wq_e = nc.dram_tensor("wq", [P, DK, C], BF16, kind="ExternalInput")
    wk_e = nc.dram_tensor("wk", [P, DK, C], BF16, kind="ExternalInput")
    wv_e = nc.dram_tensor("wv", [P, DK, C], BF16, kind="ExternalInput")
    wo_e = nc.dram_tensor("wo", [P, 2, D], BF16, kind="ExternalInput")
    # two partial outputs (one per head-pair); host sums them while unsharding
    out_e = nc.dram_tensor("out", [N, D], F32, kind="ExternalOutput")
    out2_e = nc.dram_tensor("out2", [N, D], F32, kind="ExternalOutput")

    from contextlib import ExitStack

    with ExitStack() as ctx:
        ps_mm = ctx.enter_context(tc.tile_pool(name="ps_mm", bufs=2, space="PSUM"))
        ps_sim = ctx.enter_context(tc.tile_pool(name="ps_sim", bufs=2, space="PSUM"))
        ps_av = ctx.enter_context(tc.tile_pool(name="ps_av", bufs=2, space="PSUM"))
        qk_pool = ctx.enter_context(tc.tile_pool(name="qk", bufs=1))
        v_pool = ctx.enter_context(tc.tile_pool(name="v", bufs=1))
        exp_pool = ctx.enter_context(tc.tile_pool(name="exp", bufs=8))
        attn_pool = ctx.enter_context(tc.tile_pool(name="attnT", bufs=4))
        small = ctx.enter_context(tc.tile_pool(name="small", bufs=4))
        out_pool = ctx.enter_context(tc.tile_pool(name="osb", bufs=4))
        tokp = ctx.enter_context(tc.tile_pool(name="tok", bufs=1))
        wp = ctx.enter_context(tc.tile_pool(name="w", bufs=1))

        # ---- input DMA: few fat transfers (one descriptor per partition),
        # all on the sync sequencer so queue order == priority order ----
        tok = [tokp.tile([P, DK, IC], BF16, tag=f"tok{ic}", name=f"tok{ic}")
               for ic in range(NIC)]
        wq_sb = wp.tile([P, DK, C], BF16, tag="wq", name="wqs")
        wk_sb = wp.tile([P, DK, C], BF16, tag="wk", name="wks")
        wv_sb = wp.tile([P, DK, C], BF16, tag="wv", name="wvs")
        wo_sb = wp.tile([P, 2, D], BF16, tag="wo", name="wos")

        # all input DMAs go through the sync sequencer: descriptors enqueue
        # in trigger order, so issuing from one engine in priority order
        # guarantees the critical path (wk + first token quarter) drains the
        # queues first. (Issuing the rest from scalar/gpsimd in parallel was
        # tried and regressed: their descriptors jump ahead of tok0's.)
        nc.sync.dma_start(out=wk_sb[:, 0:2, :], in_=wk_e[:, 0:2, :])
        nc.sync.dma_start(out=tok[0][:, 0:2, :], in_=tok_e[0, :, 0:2, :])
        nc.sync.dma_start(out=wk_sb[:, 2:4, :], in_=wk_e[:, 2:4, :])
        nc.sync.dma_start(out=tok[0][:, 2:4, :], in_=tok_e[0, :, 2:4, :])
        nc.sync.dma_start(out=wk_sb[:, 4:8, :], in_=wk_e[:, 4:8, :])
        nc.sync.dma_start(out=tok[0][:, 4:8, :], in_=tok_e[0, :, 4:8, :])
        nc.sync.dma_start(out=wq_sb[:], in_=wq_e[:])
        nc.sync.dma_start(out=tok[1][:], in_=tok_e[1])
        nc.sync.dma_start(out=tok[2][:], in_=tok_e[2])
        nc.sync.dma_start(out=tok[3][:], in_=tok_e[3])
        nc.sync.dma_start(out=wv_sb[:], in_=wv_e[:])
        nc.sync.dma_start(out=wo_sb[:], in_=wo_e[:])

        # qT/kT[p]: rows 0-63 head 2p, rows 64-127 head 2p+1
        qT = [qk_pool.tile([P, N], BF16, tag=f"qT{p}", name=f"qT{p}")
              for p in range(2)]
        kT = [qk_pool.tile([P, N], BF16, tag=f"kT{p}", name=f"kT{p}")
              for p in range(2)]
        vtile = v_pool.tile([P, JT, HPC, HD + 1], BF16, tag="v", name="vtile")
        attnT = [[attn_pool.tile([P, IC2], BF16, tag=f"attnT{p}_{c2}",
                                 name=f"attnT{p}_{c2}")
                  for c2 in range(NIC2)] for p in range(2)]

        bg = []  # deferred PE work (one matmul per thunk), drained in attn loops
        last_mm = [None]  # last attention matmul, for tail-wo ordering deps

        def drain_bg(n):
            for _ in range(n):
                if bg:
                    bg.pop(0)()

        def emit_proj(p, ic, w_sb, dst, defer=False):
            state = {}

            def mk(dk, state):
                def thunk():
                    if dk == 0:
                        state["ps"] = ps_mm.tile([P, IC], F32, tag="mm", name="ps")
                    nc.tensor.matmul(
                        state["ps"][:],
                        lhsT=w_sb[:, dk, P * p:P * (p + 1)],
                        rhs=tok[ic][:, dk, :],
                        start=(dk == 0),
                        stop=(dk == DK - 1),
                    )
                    if dk == DK - 1:
                        nc.vector.tensor_copy(
                            dst[:, IC * ic:IC * (ic + 1)], state["ps"][:])
                return thunk

            for dk in range(DK):
                t = mk(dk, state)
                if defer:
                    bg.append(t)
                else:
                    t()

        def emit_v(jt, defer=False):
            state = {}

            def mk_v(dk, state):
                def thunk():
                    if dk == 0:
                        state["ps"] = ps_mm.tile(
                            [P, HPC, HD], F32, tag="mm", name="ps")
                    nc.tensor.matmul(
                        state["ps"][:],
                        lhsT=tok[jt // 4][:, dk, P * (jt % 4):P * (jt % 4 + 1)],
                        rhs=wv_sb[:, dk, :],
                        start=(dk == 0),
                        stop=(dk == DK - 1),
                    )
                    if dk == DK - 1:
                        nc.vector.tensor_copy(
                            vtile[:, jt, :, 0:HD], state["ps"][:])
                return thunk

            for dk in range(DK):
                t = mk_v(dk, state)
                if defer:
                    bg.append(t)
                else:
                    t()

        def emit_attn(p, c2, q, drain=4, pre_norm_cb=None, half_cb=None):
            h = 2 * p + q  # local head index 0..3
            r0 = HD * q    # partition row base inside the pair tiles
            # two av accumulators, one per 512-wide half of this chunk
            avp = [ps_av.tile([HD + 1, IC], F32, tag="av", name="avp")
                   for _ in range(2)]
            ets = [None] * JT

            def av_mm(k, stop):
                for half in range(2):
                    mm = nc.tensor.matmul(
                        avp[half][:],
                        lhsT=vtile[:, k, h, :],
                        rhs=ets[k][:, IC * half:IC * (half + 1)],
                        start=(k == 0),
                        stop=stop,
                    )
                    last_mm[0] = mm
                ets[k] = None

            for jt in range(JT):
                sp = ps_sim.tile([P, IC2], F32, tag="sim", name="sp")
                # two F=512 matmuls: a single F=1024 psum write spanning two
                # banks is rejected by codegen
                for half in range(2):
                    nc.tensor.matmul(
                        sp[:, IC * half:IC * (half + 1)],
                        lhsT=kT[p][r0:r0 + HD, P * jt:P * (jt + 1)],
                        rhs=qT[p][r0:r0 + HD,
                                  IC2 * c2 + IC * half:IC2 * c2 + IC * (half + 1)],
                        start=True,
                        stop=True,
                    )
                et = exp_pool.tile([P, IC2], BF16, tag="exp", name="et")
                nc.scalar.activation(et[:], sp[:], AF.Exp)
                ets[jt] = et
                drain_bg(drain)
                if jt >= 3:
                    av_mm(jt - 3, stop=False)
            av_mm(JT - 3, stop=False)
            av_mm(JT - 2, stop=False)
            av_mm(JT - 1, stop=True)
            if pre_norm_cb is not None:
                pre_norm_cb()
            for half in range(2):
                # reciprocal_approx_fast cannot read PSUM (wrong results were
                # measured) - stage the ones-row sums through SBUF first
                sums = small.tile([1, IC], F32, tag="sums", name="sums")
                nc.vector.tensor_copy(sums[:], avp[half][HD:HD + 1, :])
                rec1 = small.tile([1, IC], F32, tag="rec1", name="rec1")
                nc.vector.reciprocal_approx_fast(out=rec1[:], in_=sums[:])
                rec64 = small.tile([HD, IC], F32, tag="rec64", name="rec64")
                nc.gpsimd.partition_broadcast(rec64[:], rec1[:])
                nc.vector.tensor_mul(
                    attnT[p][c2][r0:r0 + HD, IC * half:IC * (half + 1)],
                    avp[half][0:HD, :],
                    rec64[:],
                )
                if half_cb is not None:
                    half_cb(half)

        def emit_wo(p, c2, i, defer=True, use_alt_psum=False, copy_eng=None,
                    final_tail=False):
            # wo partial for pair p, seq tile i (128 rows) of chunk c2,
            # written straight to this pair's dram partial. copy_eng lets
            # tail thunks route their psum->sbuf copies to the scalar engine
            # so they never queue ahead of the softmax-normalize chain on
            # DVE. final_tail additionally splits the copies across scalar
            # and vector and the output DMA across sync and scalar queues to
            # shorten the post-last-matmul serial chain.
            dst = out_e if p == 0 else out2_e
            nt = 8 * c2 + i
            if copy_eng == "scalar":
                copier = nc.scalar.copy
            else:
                copier = nc.vector.tensor_copy

            def thunk():
                pss = []
                for do in range(2):
                    if use_alt_psum and do == 1:
                        ps = ps_sim.tile([P, IC], F32, tag="sim", name="ps")
                    else:
                        ps = ps_mm.tile([P, IC], F32, tag="mm", name="ps")
                    mm = nc.tensor.matmul(
                        ps[:],
                        lhsT=attnT[p][c2][:, P * i:P * (i + 1)],
                        rhs=wo_sb[:, p, IC * do:IC * (do + 1)],
                        start=True,
                        stop=True,
                    )
                    if last_mm[0] is not None:
                        add_dep_helper(
                            mm.ins, last_mm[0].ins, sync=False,
                            reason="keep wo behind attention in PE order")
                    pss.append(ps)
                osb = out_pool.tile([P, D], F32, tag="osb", name="osb")
                if final_tail:
                    nc.scalar.copy(osb[:, 0:IC], pss[0][:])
                    nc.vector.tensor_copy(osb[:, IC:D], pss[1][:])
                    nc.sync.dma_start(
                        out=dst[P * nt:P * (nt + 1), 0:IC], in_=osb[:, 0:IC])
                    nc.scalar.dma_start(
                        out=dst[P * nt:P * (nt + 1), IC:D], in_=osb[:, IC:D])
                else:
                    for do in range(2):
                        copier(osb[:, IC * do:IC * (do + 1)], pss[do][:])
                    nc.sync.dma_start(out=dst[P * nt:P * (nt + 1), :], in_=osb[:])

            if defer:
                bg.append(thunk)
            else:
                thunk()

        # ---- emission schedule ----
        # lead-in ordered to match DMA arrival: K ic0 (wk+tok0), Q ic0 (wq),
        # then K/Q as later token quarters land. Attention pair0/c2=0 needs
        # all of K pair0 + Q pair0 chunks 0-1; everything else is deferred
        # into the attention drain loops.
        emit_proj(0, 0, wk_sb, kT[0])
        emit_proj(0, 0, wq_sb, qT[0])
        emit_proj(0, 1, wk_sb, kT[0])
        emit_proj(0, 1, wq_sb, qT[0])
        emit_proj(0, 2, wk_sb, kT[0])
        emit_proj(0, 3, wk_sb, kT[0])
        emit_proj(0, 2, wq_sb, qT[0], defer=True)
        emit_proj(0, 3, wq_sb, qT[0], defer=True)

        nc.vector.memset(vtile[:, :, :, HD:HD + 1], 1.0)
        for jt in range(JT):
            emit_v(jt, defer=(jt >= 3))

        for ic in range(NIC):
            emit_proj(1, ic, wk_sb, kT[1], defer=True)
        for ic in range(NIC):
            emit_proj(1, ic, wq_sb, qT[1], defer=True)

        # drain=10 in the first phase keeps V-tile production ahead of the
        # lag-3 AV consumer.
        emit_attn(0, 0, 0, drain=10)
        emit_attn(0, 0, 1, drain=3)
        for i in range(8):
            emit_wo(0, 0, i)
        emit_attn(0, 1, 0, drain=3)
        emit_attn(0, 1, 1, drain=3)
        for i in range(8):
            emit_wo(0, 1, i)
        emit_attn(1, 0, 0, drain=3)
        emit_attn(1, 0, 1, drain=3)
        for i in range(4):
            emit_wo(1, 0, i)
        emit_attn(1, 1, 0, drain=3)

        # last phase: wo(1,0) tiles 4-7 were held back as ready-to-run PE
        # work to fill the softmax-chain latency after the final AV, and
        # each half's wo(1,1) tiles start as soon as its normalize lands.
        # Tail copies that precede a later DVE normalize go via the scalar
        # engine (idle once the last exp is done) to keep DVE clear.
        def pre_norm():
            drain_bg(len(bg))
            for i in range(4, 8):
                emit_wo(1, 0, i, defer=False, copy_eng="scalar")

        def last_half(half):
            ce = "scalar" if half == 0 else None
            for i in range(4 * half, 4 * half + 4):
                emit_wo(1, 1, i, defer=False, use_alt_psum=True, copy_eng=ce,
                        final_tail=(half == 1 and i >= 6))

        emit_attn(1, 1, 1, drain=2, pre_norm_cb=pre_norm, half_cb=last_half)


def build():
    if "nc" not in _compiled:
        nc = bacc.Bacc("TRN2", target_bir_lowering=False, debug=False,
                       num_devices=NCORES)
        with tile.TileContext(nc) as tc:
            _emit(tc)
        nc.compile()
        _compiled["nc"] = nc
    return _compiled["nc"]


def kernel(tokens, context_mask, Wq, Wkv, Wo, _profile=False):
    tokens = np.asarray(tokens, dtype=np.float32)
    Wq = np.asarray(Wq, dtype=np.float32)
    Wkv = np.asarray(Wkv, dtype=np.float32)
    Wo = np.asarray(Wo, dtype=np.float32)

    nc = build()
    scale = np.float32(HD ** -0.5)

    # tokens[b].T tiled to [ic, p, dk, 512] so each quarter-chunk DMA is one
    # fat contiguous descriptor per partition.
    tokH = []
    for b in range(B):
        t4 = np.ascontiguousarray(tokens[b].T).astype(NPBF16)
        t4 = t4.reshape(DK, P, NIC, IC).transpose(2, 1, 0, 3)
        tokH.append(np.ascontiguousarray(t4))

    def wtile(w):  # [D, C] -> [p, dk, c]
        return np.ascontiguousarray(
            w.astype(NPBF16).reshape(DK, P, C).transpose(1, 0, 2))

    in_maps = []
    for core in range(NCORES):
        b, g = divmod(core, G)
        wo_slice = Wo[C * g:C * (g + 1), :].astype(NPBF16)
        in_maps.append({
            "tok": tokH[b],
            "wq": wtile(Wq[:, C * g:C * (g + 1)] * scale),
            "wk": wtile(Wkv[:, C * g:C * (g + 1)]),
            "wv": wtile(Wkv[:, D + C * g:D + C * (g + 1)]),
            "wo": np.ascontiguousarray(
                wo_slice.reshape(2, P, D).transpose(1, 0, 2)),
        })
    kwargs = {}
    if _profile:
        kwargs = dict(trace=True,
                      tmpdir=_profile if isinstance(_profile, str) else None)
    res = run_bass_kernel_spmd(nc, in_maps, core_ids=list(range(NCORES)), **kwargs)

    out = np.zeros((B, N, D), dtype=np.float32)
    for core in range(NCORES):
        b = core // G
        out[b] += res.results[core]["out"].astype(np.float32)
        out[b] += res.results[core]["out2"].astype(np.float32)
    if _profile:
        return out, res
    return out
